# revision 53
# baseline (speedup 1.0000x reference)
"""BasicTransformerBlock on 8 TRN2 NeuronCores.

Strategy: pure data parallelism over batch (B=8, one batch element per core).
v2: all-bf16 weights/activations (fp32 PSUM + LN row math), host-pretiled
weights for contiguous DMA, gpsimd partition_broadcast instead of DRAM
roundtrips, rstd via ln/exp (stays in the softmax-exp ACT table set), paired
[128,1024] exps over both token chunks, FF2 accumulation interleaved with FF1,
manual PSUM bank plan, one-behind matmul ordering to keep the PE FIFO dense.
"""

import math

import numpy as np
import ml_dtypes

import concourse.bass as bass
import concourse.mybir as mybir
import concourse.tile as tile
from concourse import bacc
from concourse.bass_utils import run_bass_kernel_spmd

# Route both Ln and Exp to the combined natural_log_exp_and_others ACT table
# set (greedy per-function selection would alternate between exp_and_others
# and natural_log, paying a ~2.6us table reload at every layernorm). Keys and
# order are preserved so act_func_set ids still match walrus's view; emptied
# sets simply can't be selected.
import concourse.hw_specs as _hw_specs

_ORIG_GAT = _hw_specs.get_activation_tables


def _gat_patched(arch):
    t = _ORIG_GAT(arch)
    if "natural_log_exp_and_others" in t:
        for k in ("exp_and_others", "natural_log", "exp_and_friends"):
            if k in t:
                t[k] = set()
    return t


bacc.get_activation_tables = _gat_patched

F32 = mybir.dt.float32
F32R = mybir.dt.float32r
BF16 = mybir.dt.bfloat16
AF = mybir.ActivationFunctionType
OP = mybir.AluOpType

P = 128
B = 8
NT = 1024          # query tokens
D = 640            # model dim; 5 chunks of 128
KC = 5
NH = 8             # heads
DH = 80            # head dim
CM = 77            # context tokens
CD = 768           # context dim; 6 chunks
CKC = 6
FH = 2560          # GEGLU half hidden; 20 chunks of 128
FJ = 20
NC = 2             # token chunks of 512
NW = 512
ISCALE = 1.0 / math.sqrt(DH)
LN_EPS = 1e-5

# broadcast mechanism: "gpsimd" | "dram"
BCAST = "gpsimd"


def _emit(nc, tc, apply_gb):
    d = nc._kd
    with (
        tc.tile_pool(name="sb", bufs=1) as sb,
        tc.tile_pool(name="ps", bufs=1, space="PSUM") as ps,
    ):
        _emit_body(nc, tc, d, sb, ps, apply_gb)


def _emit_body(nc, tc, d, sb, ps, apply_gb):
    # one big PSUM tensor; manual bank plan, Tile tracks deps per bank
    pb = ps.tile([P, 8, NW], F32, name="pb")

    def bank(i):
        return pb[:, i, :]

    def bankpair(i):
        return pb[:, i:i + 2, :].rearrange("p a n -> p (a n)")

    # ---------------- loads (3 queues, in use order) ----------------
    # x on sync+gpsimd, first weights on scalar, so the critical first
    # projection group unblocks as early as possible
    xTb = sb.tile([P, KC, NT], BF16, tag="xTb", name="xTb")
    for c, q in zip(range(KC), (nc.sync, nc.sync, nc.sync, nc.gpsimd,
                                nc.gpsimd)):
        q.dma_start(xTb[:, c, :], d["xTb"][:, c, :])

    def wload(key, shape, name, tag, bufs=1):
        t = sb.tile(shape, BF16, tag=tag, bufs=bufs, name=name)
        nc.sync.dma_start(t, d[key][:, :, :])
        return t

    w_sa_q = sb.tile([P, KC, D], BF16, tag="w640", bufs=4, name="w_sa_q")
    nc.scalar.dma_start(w_sa_q, d["sa_wq_t"][:, :, :])
    w_sa_k = sb.tile([P, KC, D], BF16, tag="w640", bufs=4, name="w_sa_k")
    nc.scalar.dma_start(w_sa_k, d["sa_wk_t"][:, :, :])
    w_sa_v = wload("sa_wv_t", [P, KC, D], "w_sa_v", "w640", 4)
    w_sa_o = wload("sa_wo_h", [DH, NH, D], "w_sa_o", "wo", 1)

    # constants / biases
    ones_bf = sb.tile([P, 1], BF16, tag="ones", name="ones_bf")
    nc.sync.dma_start(ones_bf, d["ones_bf"][:, :])
    epsd2 = sb.tile([1, 1], F32, tag="epsd2", name="epsd2")
    nc.sync.dma_start(epsd2, d["epsd2"][:, :])
    b_sa_bo = sb.tile([P, KC], F32, tag="b1", name="b_sa_bo")
    nc.sync.dma_start(b_sa_bo, d["sa_bo_p"][:, :])
    b_ca_bo = sb.tile([P, KC], F32, tag="b2", name="b_ca_bo")
    nc.sync.dma_start(b_ca_bo, d["ca_bo_p"][:, :])
    b_ff2 = sb.tile([P, KC], F32, tag="b3", name="b_ff2")
    nc.sync.dma_start(b_ff2, d["ff_b2_p"][:, :])
    b_f1a = sb.tile([P, FJ], F32, tag="b4", name="b_f1a")
    nc.sync.dma_start(b_f1a, d["ff_b1a_p"][:, :])
    b_f1g = sb.tile([P, FJ], F32, tag="b5", name="b_f1g")
    nc.sync.dma_start(b_f1g, d["ff_b1g_p"][:, :])
    lngb = {}
    if apply_gb:
        for ln in (1, 2, 3):
            for gb in ("g", "b"):
                t = sb.tile([P, KC], F32, tag=f"ln{ln}{gb}", name=f"ln{ln}{gb}")
                nc.sync.dma_start(t, d[f"ln{ln}_{gb}_p"][:, :])
                lngb[(ln, gb)] = t

    ctxT = sb.tile([P, CKC, CM], BF16, tag="ctxT", name="ctxT")
    nc.sync.dma_start(ctxT, d["ctxT_bf"][:, :, :])
    w_ca_k = sb.tile([P, CKC, D], BF16, tag="w768", bufs=2, name="w_ca_k")
    nc.sync.dma_start(w_ca_k, d["ca_wk_t"][:, :, :])
    w_ca_v = sb.tile([P, CKC, D], BF16, tag="w768", bufs=2, name="w_ca_v")
    nc.sync.dma_start(w_ca_v, d["ca_wv_t"][:, :, :])
    w_ca_q = wload("ca_wq_t", [P, KC, D], "w_ca_q", "w640", 4)

    if BCAST == "dram":
        zdram = nc.dram_tensor("zdram", [64, NT], F32)
        zslot = [0]

    def bcast_row(row, out_parts, width, name, dt=F32):
        """row: [1, width] SBUF -> [out_parts, width] SBUF (same dtype)."""
        out = sb.tile([out_parts, width], dt, tag="bc_out", bufs=2, name=name)
        if BCAST == "gpsimd":
            nc.gpsimd.partition_broadcast(out, row)
        else:
            s = zslot[0]
            zslot[0] += 1
            nc.sync.dma_start(zdram[s:s + 1, 0:width], row)
            nc.sync.dma_start(
                out, zdram[s:s + 1, 0:width].to_broadcast((out_parts, width)))
        return out

    # ---------------- SA: Q/K projections (one head ahead) ----------------
    def qk_proj(h, w_q, w_k, src):
        hs = slice(h * DH, (h + 1) * DH)
        qt = sb.tile([DH, NT], BF16, tag="qk", bufs=4, name=f"qt_s{h}")
        kt = sb.tile([DH, NT], BF16, tag="qk", bufs=4, name=f"kt_s{h}")
        # all-Q before all-K: the first groups only need w_q
        for w, b, dest in ((w_q, 2, qt), (w_k, 3, kt)):
            for ncq in range(NC):
                ncs = slice(ncq * NW, (ncq + 1) * NW)
                pq = bank(b)
                for c in range(KC):
                    nc.tensor.matmul(pq[0:DH, :], w[:, c, hs], src[:, c, ncs],
                                     start=(c == 0), stop=(c == KC - 1))
                nc.vector.tensor_copy(dest[:, ncs], pq[0:DH, :])
        return qt, kt

    qk = [None] * NH
    qk[0] = qk_proj(0, w_sa_q, w_sa_k, xTb)
    qk[1] = qk_proj(1, w_sa_q, w_sa_k, xTb)

    # ---------------- SA: V projection into V_aug (token-major) ----------
    v_aug = sb.tile([P, NH, NH, 97], BF16, tag="vaug", name="v_aug")
    nc.vector.memset(v_aug[:, :, :, 80:96], 0.0)
    nc.vector.memset(v_aug[:, :, :, 96:97], 1.0)
    for tch in range(NH):
        for g in range(2):
            pv = bank(4 + (2 * tch + g) % 4)
            for c in range(KC):
                nc.tensor.matmul(
                    pv[:, 0:320], xTb[:, c, tch * P:(tch + 1) * P],
                    w_sa_v[:, c, g * 320:(g + 1) * 320],
                    start=(c == 0), stop=(c == KC - 1))
            nc.vector.tensor_copy(
                v_aug[:, tch, 4 * g:4 * g + 4, 0:80],
                pv[:, 0:320].rearrange("p (s e) -> p s e", e=80))

    # ---------------- attention inner (paired exp over both ncq) ----------
    def attn_head(h, qt, kt_sl, vaug_sl, o_tile, mchunks, mpart, pref,
                  po_banks=(0, 1)):
        po = [bank(po_banks[0]), bank(po_banks[1])]
        ets = []
        for mc in range(mchunks):
            psc = bankpair(4 + 2 * (mc % 2))
            for ncq in range(NC):
                nc.tensor.matmul(
                    psc[0:mpart, ncq * NW:(ncq + 1) * NW], kt_sl(mc),
                    qt[:, ncq * NW:(ncq + 1) * NW], start=True, stop=True)
            et = sb.tile([mpart, NT], BF16, tag="epool", bufs=2,
                         name=f"e{pref}_{h}_{mc}")
            nc.scalar.activation(et, psc[0:mpart, :], AF.Exp, scale=ISCALE)
            ets.append(et)
            if mc >= 1:
                for ncq in range(NC):
                    nc.tensor.matmul(
                        po[ncq][0:97, :], vaug_sl(mc - 1),
                        ets[mc - 1][:, ncq * NW:(ncq + 1) * NW],
                        start=(mc == 1), stop=False, skip_group_check=True)
        last = mchunks - 1
        for ncq in range(NC):
            nc.tensor.matmul(
                po[ncq][0:97, :], vaug_sl(last),
                ets[last][:, ncq * NW:(ncq + 1) * NW],
                start=(mchunks == 1), stop=True, skip_group_check=True)
        # softmax denominator -> broadcast -> reciprocal -> normalize
        zrow = sb.tile([1, NT], F32, tag="zrow", bufs=2, name=f"zr{pref}_{h}")
        for ncq in range(NC):
            nc.vector.tensor_copy(
                zrow[:, ncq * NW:(ncq + 1) * NW], po[ncq][96:97, :])
        zb = bcast_row(zrow, DH, NT, f"zb{pref}_{h}", dt=F32)
        nc.vector.reciprocal_approx_fast(zb, zb)
        for ncq in range(NC):
            ncs = slice(ncq * NW, (ncq + 1) * NW)
            nc.vector.tensor_tensor(o_tile[:, h, ncs], po[ncq][0:DH, :],
                                    zb[:, ncs], OP.mult)

    o_sa = sb.tile([DH, NH, NT], BF16, tag="opool", bufs=2, name="o_sa")
    for h in range(NH):
        if h + 1 < NH and qk[h + 1] is None:
            qk[h + 1] = qk_proj(h + 1, w_sa_q, w_sa_k, xTb)
        qt, kt = qk[h]
        attn_head(
            h, qt,
            kt_sl=lambda mc, _kt=kt: _kt[:, mc * P:(mc + 1) * P],
            vaug_sl=lambda mc, _h=h: v_aug[:, mc, _h, :],
            o_tile=o_sa, mchunks=NH, mpart=P, pref="s")
        qk[h] = None

    # late weight loads (sync FIFO: slots free mid-SA; needed post-SA)
    w_ca_o = wload("ca_wo_h", [DH, NH, D], "w_ca_o", "wo", 1)

    # ---------------- CA K projections (fill the SA->out_proj PE gap) ----
    kt_ca = sb.tile([DH, NH, CM], BF16, tag="ktca", name="kt_ca")
    for h in range(NH):
        hs = slice(h * DH, (h + 1) * DH)
        pk = bank(2 + h % 2)
        for c in range(CKC):
            nc.tensor.matmul(pk[0:DH, 0:CM], w_ca_k[:, c, hs], ctxT[:, c, :],
                             start=(c == 0), stop=(c == CKC - 1))
        nc.vector.tensor_copy(kt_ca[:, h, :], pk[0:DH, 0:CM])

    # ---------------- residual + LN ----------------
    def resid_tile(name):
        return sb.tile([P, KC, NT], BF16, tag="resid", bufs=2, name=name)

    def psum_bias_add(pr, bias_ap, res_in_ap, out_ap, name, w=NW):
        """out = (pr + bias) + res_in, split ACT (psum read) + bf16 DVE add."""
        tmp = sb.tile([P, w], BF16, tag="optmp", bufs=3, name=name)
        nc.scalar.activation(tmp, pr, AF.Identity, bias=bias_ap)
        nc.vector.tensor_tensor(out_ap, tmp, res_in_ap, OP.add)

    def out_proj(ncq, wo_t, o_tile, bo_t, res_in, res_out, pref):
        ncs = slice(ncq * NW, (ncq + 1) * NW)
        for do in range(KC):
            dos = slice(do * P, (do + 1) * P)
            pr = bank(4 + do % 4)
            for h in range(NH):
                nc.tensor.matmul(pr, wo_t[:, h, dos], o_tile[:, h, ncs],
                                 start=(h == 0), stop=(h == NH - 1))
            psum_bias_add(pr, bo_t[:, do:do + 1], res_in[:, do, ncs],
                          res_out[:, do, ncs], f"tm{pref}_{ncq}_{do}")

    def layernorm_nc(rT, ln_idx, ncq, out_bf=None, out_writer=None):
        """Feature-major LN of rT[:, :, ncs] (bf16).

        Either writes bf16 out_bf[:, c, ncs], or calls
        out_writer(c, f32_chunk_ap) per chunk (LN3/store path).
        Row math reads the stat PSUMs directly (no row copies); the packed
        [rstd/D | mu*rstd] row is bf16 -> one cheap broadcast, 16-bit
        normalize ops.
        """
        ncs = slice(ncq * NW, (ncq + 1) * NW)
        psum_s = bank(2)[0:1, :]
        psum_q = bank(3)[0:1, :]
        for c0, cw in ((0, 2), (2, 2), (4, 1)):
            sq = sb.tile([P, 2, NW], BF16, tag="sq", bufs=2,
                         name=f"sq_{ln_idx}_{ncq}_{c0}")
            nc.vector.tensor_tensor(sq[:, 0:cw, :], rT[:, c0:c0 + cw, ncs],
                                    rT[:, c0:c0 + cw, ncs], OP.mult)
            for c in range(c0, c0 + cw):
                nc.tensor.matmul(psum_s, ones_bf, rT[:, c, ncs],
                                 start=(c == 0), stop=(c == KC - 1))
                nc.tensor.matmul(psum_q, ones_bf, sq[:, c - c0, :],
                                 start=(c == 0), stop=(c == KC - 1))
        # var*D^2 = q*D - s^2 ; rstd/D = exp(-0.5*ln(var*D^2 + eps*D^2))
        t2 = sb.tile([1, NW], F32, tag="lrow", bufs=3, name=f"t2_{ln_idx}_{ncq}")
        nc.scalar.activation(t2, psum_s, AF.Square)
        vrow = sb.tile([1, NW], F32, tag="lrow", bufs=3, name=f"vr_{ln_idx}_{ncq}")
        nc.vector.scalar_tensor_tensor(
            out=vrow, in0=psum_q, scalar=float(D), in1=t2,
            op0=OP.mult, op1=OP.subtract)
        nc.scalar.activation(vrow, vrow, AF.Ln, bias=epsd2)
        # pack [rstd/D | mu*rstd] into one bf16 row -> single broadcast
        packed = sb.tile([1, 2 * NW], BF16, tag="lrow2", bufs=2,
                         name=f"pk_{ln_idx}_{ncq}")
        nc.scalar.activation(packed[:, 0:NW], vrow, AF.Exp, scale=-0.5)
        nc.vector.tensor_tensor(packed[:, NW:2 * NW], psum_s, packed[:, 0:NW],
                                OP.mult)
        mb = bcast_row(packed, P, 2 * NW, f"mb_{ln_idx}_{ncq}", dt=BF16)
        rD_b = mb[:, 0:NW]
        ms_b = mb[:, NW:2 * NW]
        for c0, cw in ((0, 2), (2, 2), (4, 1)):
            rD_w = rD_b.unsqueeze(1).to_broadcast((P, cw, NW))
            ms_w = ms_b.unsqueeze(1).to_broadcast((P, cw, NW))
            t1 = sb.tile([P, 2, NW], BF16, tag="t1_b", bufs=2,
                         name=f"t1_{ln_idx}_{ncq}_{c0}")
            nc.vector.scalar_tensor_tensor(
                out=t1[:, 0:cw, :], in0=rT[:, c0:c0 + cw, ncs],
                scalar=float(D), in1=rD_w, op0=OP.mult, op1=OP.mult)
            if out_bf is not None and not apply_gb:
                nc.vector.tensor_tensor(out_bf[:, c0:c0 + cw, ncs],
                                        t1[:, 0:cw, :], ms_w, OP.subtract)
            elif out_bf is not None:
                t0 = sb.tile([P, 2, NW], BF16, tag="t0_b", bufs=2,
                             name=f"t0_{ln_idx}_{ncq}_{c0}")
                nc.vector.tensor_tensor(t0[:, 0:cw, :], t1[:, 0:cw, :], ms_w,
                                        OP.subtract)
                for c in range(c0, c0 + cw):
                    nc.vector.tensor_scalar(
                        out=out_bf[:, c, ncs], in0=t0[:, c - c0, :],
                        scalar1=lngb[(ln_idx, "g")][:, c:c + 1],
                        scalar2=lngb[(ln_idx, "b")][:, c:c + 1],
                        op0=OP.mult, op1=OP.add)
            else:
                outc = sb.tile([P, 2, NW], F32, tag="outc", bufs=2,
                               name=f"oc_{ncq}_{c0}")
                nc.vector.tensor_tensor(outc[:, 0:cw, :], t1[:, 0:cw, :],
                                        ms_w, OP.subtract)
                if apply_gb:
                    for c in range(c0, c0 + cw):
                        nc.vector.tensor_scalar(
                            out=outc[:, c - c0, :], in0=outc[:, c - c0, :],
                            scalar1=lngb[(ln_idx, "g")][:, c:c + 1],
                            scalar2=lngb[(ln_idx, "b")][:, c:c + 1],
                            op0=OP.mult, op1=OP.add)
                for c in range(c0, c0 + cw):
                    out_writer(c, outc[:, c - c0, :])

    # ---------------- SA out-proj + LN1 ----------------
    r1T = resid_tile("r1T")
    x1b = resid_tile("x1b")
    for ncq in range(NC):
        out_proj(ncq, w_sa_o, o_sa, b_sa_bo, xTb, r1T, 's')
        layernorm_nc(r1T, 1, ncq, out_bf=x1b)

    # vca + the ncq0 halves of the CA q projections fill the PE while the
    # LN1 row math / broadcast completes (they only need x1b[ncq0]).
    vca_aug = sb.tile([CM, NH, 97], BF16, tag="vca", name="vca_aug")
    nc.vector.memset(vca_aug[:, :, 80:96], 0.0)
    nc.vector.memset(vca_aug[:, :, 96:97], 1.0)
    for g in range(2):
        pv = bank(2 + g)
        for c in range(CKC):
            nc.tensor.matmul(pv[0:CM, 0:320], ctxT[:, c, :],
                             w_ca_v[:, c, g * 320:(g + 1) * 320],
                             start=(c == 0), stop=(c == CKC - 1))
        nc.vector.tensor_copy(
            vca_aug[:, 4 * g:4 * g + 4, 0:80],
            pv[0:CM, 0:320].rearrange("p (s e) -> p s e", e=80))

    # ---------------- CA: per-head Q + attention ----------------
    o_ca = sb.tile([DH, NH, NT], BF16, tag="opool", bufs=2, name="o_ca")

    def ca_qproj_half(h, ncq, qt):
        hs = slice(h * DH, (h + 1) * DH)
        ncs = slice(ncq * NW, (ncq + 1) * NW)
        pq = bank(2 + ncq)
        for c in range(KC):
            nc.tensor.matmul(pq[0:DH, :], w_ca_q[:, c, hs], x1b[:, c, ncs],
                             start=(c == 0), stop=(c == KC - 1))
        nc.vector.tensor_copy(qt[:, ncs], pq[0:DH, :])

    def ca_qt(h):
        return sb.tile([DH, NT], BF16, tag="qkc", bufs=4, name=f"qt_c{h}")

    qt_ca = [None] * NH
    for h in range(4):
        qt_ca[h] = ca_qt(h)
        ca_qproj_half(h, 0, qt_ca[h])
    ca_qproj_half(0, 1, qt_ca[0])
    for h in range(NH):
        if h + 1 < 4:
            ca_qproj_half(h + 1, 1, qt_ca[h + 1])
        elif h + 1 < NH:
            qt_ca[h + 1] = ca_qt(h + 1)
            ca_qproj_half(h + 1, 0, qt_ca[h + 1])
            ca_qproj_half(h + 1, 1, qt_ca[h + 1])
        attn_head(
            h, qt_ca[h],
            kt_sl=lambda mc, _h=h: kt_ca[:, _h, :],
            vaug_sl=lambda mc, _h=h: vca_aug[:, _h, :],
            o_tile=o_ca, mchunks=1, mpart=CM, pref="c",
            po_banks=(0, 1) if h % 2 == 0 else (6, 7))
        qt_ca[h] = None

    # ---------------- CA out-proj + LN2 ----------------
    r2T = resid_tile("r2T")
    x2b = resid_tile("x2b")
    for ncq in range(NC):
        out_proj(ncq, w_ca_o, o_ca, b_ca_bo, x1b, r2T, 'c')
        layernorm_nc(r2T, 2, ncq, out_bf=x2b)

    # ---------------- FF (GEGLU), FF2 interleaved ----------------
    w_ff2 = []
    for t in range(4):
        wt = sb.tile([P, KC, D], BF16, tag="w640", bufs=4, name=f"w_ff2_{t}")
        nc.sync.dma_start(wt, d["ff_w2_t"][t])
        w_ff2.append(wt)

    r3T = resid_tile("r3T")
    store_q = [nc.gpsimd, nc.sync, nc.scalar]

    def emit_ln3(ncq):
        ncs = slice(ncq * NW, (ncq + 1) * NW)

        def store_chunk(c, outc, _ncs=ncs, _ncq=ncq):
            store_q[(_ncq * KC + c) % 3].dma_start(
                d["outT"].rearrange("(c p) n -> p c n", p=P)[:, c, _ncs],
                outc)

        layernorm_nc(r3T, 3, ncq, out_writer=store_chunk)

    pending_ln3 = None
    for ncq in range(NC):
        ncs = slice(ncq * NW, (ncq + 1) * NW)
        mprev = None
        for j in range(FJ):
            wj = sb.tile([P, 2, KC, P], BF16, tag="wff1", bufs=3,
                         name=f"wj_{ncq}_{j}")
            nc.sync.dma_start(
                wj.rearrange("p a c q -> p (a c q)"), d["ff_w1t"][j])
            pa = bank(5 + (2 * j) % 3)
            pg = bank(5 + (2 * j + 1) % 3)
            for c in range(KC):
                nc.tensor.matmul(pa, wj[:, 0, c, :], x2b[:, c, ncs],
                                 start=(c == 0), stop=(c == KC - 1))
            for c in range(KC):
                nc.tensor.matmul(pg, wj[:, 1, c, :], x2b[:, c, ncs],
                                 start=(c == 0), stop=(c == KC - 1))
            if j == 0 and pending_ln3 is not None:
                # previous chunk's LN3 stats go behind this chunk's first
                # FF1 matmuls so the PE FIFO never waits on the r3 DVE ops
                emit_ln3(pending_ln3)
                pending_ln3 = None
            gj = sb.tile([P, NW], BF16, tag="gelu", bufs=2, name=f"gj_{ncq}_{j}")
            nc.scalar.activation(gj, pg, AF.Gelu, bias=b_f1g[:, j:j + 1])
            pab = sb.tile([P, NW], BF16, tag="pab", bufs=2, name=f"pab_{ncq}_{j}")
            nc.scalar.activation(pab, pa, AF.Identity, bias=b_f1a[:, j:j + 1])
            mj = sb.tile([P, NW], BF16, tag="mfull", bufs=2, name=f"m_{ncq}_{j}")
            nc.vector.tensor_tensor(mj, pab, gj, OP.mult)
            if mprev is not None:
                jp = j - 1
                for do in range(KC):
                    nc.tensor.matmul(
                        bank(do), w_ff2[jp // KC][:, jp % KC,
                                                  do * P:(do + 1) * P],
                        mprev, start=(jp == 0), stop=False,
                        skip_group_check=True)
            mprev = mj
        for do in range(KC):
            nc.tensor.matmul(
                bank(do), w_ff2[3][:, 4, do * P:(do + 1) * P],
                mprev, start=False, stop=True, skip_group_check=True)
        for do in range(KC):
            psum_bias_add(bank(do), b_ff2[:, do:do + 1], x2b[:, do, ncs],
                          r3T[:, do, ncs], f"tmf_{ncq}_{do}")
        pending_ln3 = ncq
    emit_ln3(pending_ln3)


def _build(apply_gb):
    nc = bacc.Bacc(None, target_bir_lowering=False)
    dt_in = [
        ("xTb", [P, KC, NT], BF16), ("ctxT_bf", [P, CKC, CM], BF16),
        ("sa_wq_t", [P, KC, D], BF16), ("sa_wk_t", [P, KC, D], BF16),
        ("sa_wv_t", [P, KC, D], BF16), ("sa_wo_h", [DH, NH, D], BF16),
        ("ca_wq_t", [P, KC, D], BF16), ("ca_wk_t", [P, CKC, D], BF16),
        ("ca_wv_t", [P, CKC, D], BF16), ("ca_wo_h", [DH, NH, D], BF16),
        ("ff_w1t", [FJ, P, 2 * KC * P], BF16), ("ff_w2_t", [4, P, KC, D], BF16),
        ("sa_bo_p", [P, KC], F32), ("ca_bo_p", [P, KC], F32),
        ("ff_b2_p", [P, KC], F32),
        ("ff_b1a_p", [P, FJ], F32), ("ff_b1g_p", [P, FJ], F32),
        ("ones_bf", [P, 1], BF16), ("epsd2", [1, 1], F32),
    ]
    if apply_gb:
        for ln in (1, 2, 3):
            dt_in.append((f"ln{ln}_g_p", [P, KC], F32))
            dt_in.append((f"ln{ln}_b_p", [P, KC], F32))
    nc._kd = {}
    for name, shape, dt in dt_in:
        nc._kd[name] = nc.declare_dram_parameter(name, shape, dt,
                                                 isOutput=False)
    nc._kd["outT"] = nc.declare_dram_parameter("outT", [D, NT], F32,
                                               isOutput=True)
    with tile.TileContext(nc) as tc:
        _emit(nc, tc, apply_gb)
    nc.compile()
    return nc


def _prep_in_maps(inputs, apply_gb):
    f32 = np.float32
    bf = ml_dtypes.bfloat16
    x = np.asarray(inputs["x"], f32)
    ctx = np.asarray(inputs["context"], f32)

    def heads(w):
        return np.ascontiguousarray(
            np.asarray(w, f32).reshape(NH, DH, D).transpose(1, 0, 2)
        ).astype(bf)

    def wtile(w, kc):
        # [kc*128, M] -> [128, kc, M]
        w = np.asarray(w, f32)
        return np.ascontiguousarray(
            w.reshape(kc, P, w.shape[1]).transpose(1, 0, 2)).astype(bf)

    def part(v, cols):
        return np.ascontiguousarray(np.asarray(v, f32).reshape(cols, P).T)

    w1 = np.asarray(inputs["ff_w1"], f32)
    wa = w1[:, :FH].reshape(KC, P, FJ, P).transpose(2, 1, 0, 3)   # j p c q
    wg = w1[:, FH:].reshape(KC, P, FJ, P).transpose(2, 1, 0, 3)
    ff_w1t = np.ascontiguousarray(
        np.stack([wa, wg], axis=2).reshape(FJ, P, 2 * KC * P)).astype(bf)
    w2 = np.asarray(inputs["ff_w2"], f32)
    ff_w2t = np.ascontiguousarray(
        w2.reshape(4, KC, P, D).transpose(0, 2, 1, 3)).astype(bf)

    shared = {
        "sa_wq_t": wtile(inputs["sa_wq"], KC),
        "sa_wk_t": wtile(inputs["sa_wk"], KC),
        "sa_wv_t": wtile(inputs["sa_wv"], KC),
        "sa_wo_h": heads(inputs["sa_wo"]),
        "ca_wq_t": wtile(inputs["ca_wq"], KC),
        "ca_wk_t": wtile(inputs["ca_wk"], CKC),
        "ca_wv_t": wtile(inputs["ca_wv"], CKC),
        "ca_wo_h": heads(inputs["ca_wo"]),
        "ff_w1t": ff_w1t,
        "ff_w2_t": ff_w2t,
        "sa_bo_p": part(inputs["sa_bo"], KC),
        "ca_bo_p": part(inputs["ca_bo"], KC),
        "ff_b2_p": part(inputs["ff_b2"], KC),
        "ff_b1a_p": part(np.asarray(inputs["ff_b1"], f32)[:FH], FJ),
        "ff_b1g_p": part(np.asarray(inputs["ff_b1"], f32)[FH:], FJ),
        "ones_bf": np.ones((P, 1), bf),
        "epsd2": np.full((1, 1), LN_EPS * D * D, f32),
    }
    if apply_gb:
        for ln in (1, 2, 3):
            shared[f"ln{ln}_g_p"] = part(inputs[f"ln{ln}_g"], KC)
            shared[f"ln{ln}_b_p"] = part(inputs[f"ln{ln}_b"], KC)
    maps = []
    for i in range(B):
        m = dict(shared)
        m["xTb"] = np.ascontiguousarray(
            x[i].T.reshape(KC, P, NT).transpose(1, 0, 2)).astype(bf)
        m["ctxT_bf"] = np.ascontiguousarray(
            ctx[i].T.reshape(CKC, P, CM).transpose(1, 0, 2)).astype(bf)
        maps.append(m)
    return maps


def _needs_gb(inputs):
    for ln in (1, 2, 3):
        if not np.allclose(np.asarray(inputs[f"ln{ln}_g"]), 1.0):
            return True
        if not np.allclose(np.asarray(inputs[f"ln{ln}_b"]), 0.0):
            return True
    return False


def _run(inputs, trace=False):
    apply_gb = _needs_gb(inputs)
    nc = _build(apply_gb)
    maps = _prep_in_maps(inputs, apply_gb)
    res = run_bass_kernel_spmd(nc, maps, core_ids=list(range(B)), trace=trace)
    out = np.stack([np.asarray(r["outT"]).T for r in res.results])
    return out.astype(np.float32), res


def kernel(**inputs):
    out, _ = _run(inputs, trace=False)
    return out
